# revision 7
# baseline (speedup 1.0000x reference)
"""ColumnParallelLinearWithDelta: GPTQ-int4 LoRA-delta matmul on 8 trn2 cores.

out[d] = x @ dequant(qweight[d], qzeros[d], scales[d]) + x @ base_weight.T

Sharding: column-parallel — out_features (4096) split into 8 slices of 512,
one per NeuronCore; x replicated. Each core computes its [8, 256, 512] slice.

Math (per core, out-col slice ns):
  W[k, n]  = s[g(k), n] * (w4[k, n] - (z4[g(k), n] + 1)),  g(k) = k // 128
  delta    = x @ W = x @ (s .* w4)  -  xs @ (s .* (z4 + 1))
  with xs[t, g] = sum_{k in g} x[t, k]   (host-precomputed group sums)
  out[d]   = delta_d + base,  base = x @ base_weight[ns, :].T

Device pipeline per adapter d (all 4 row-chunks rc batched in one free dim):
  - DMA packed qweight as int16 [128, 4096] (free = rc*1024 + 2n + e)
  - 4x tensor_scalar (>> 4*sh) & 0xF -> int16 nibble planes (4x DVE mode;
    int16 halfword h=2n+e holds nibbles j = 4e + sh)
  - 4x tensor_tensor multiply by scale tile s2 (partition-replicated, x2
    free-interleaved; host-prepped) -> scaled fp16 weights (2x DVE mode)
  - 64 matmuls sh-major (pipelines with dequant): stationary xT tile for
    sub-chunk (rc, j=4e+sh) x moving plane slice (stride-2 free AP),
    accumulating into PSUM f32 per t-half
Then per adapter: K=32 correction matmul (-xs^T x sz) and an identity
matmul adding the shared base output; ScalarE copies PSUM->SBUF, DMA out.
"""

import numpy as np

# ---- problem constants (hardcoded; kernel.py must be self-contained) ----
T = 256          # tokens
IN = 4096        # in_features
OUT = 4096       # out_features
D = 8            # adapters
GROUP = 128      # quant group size
G = IN // GROUP  # 32 groups
NCORES = 8
NC_OUT = OUT // NCORES   # 512 out cols per core
RC = 4                   # row chunks of 128 packed int32 rows (512 rows total)

_PROGRAM_CACHE: dict = {}


def _build_program():
    import concourse.bacc as bacc
    import concourse.mybir as mybir
    import concourse.tile as tile
    from concourse.masks import make_identity

    nc = bacc.Bacc("TRN2", target_bir_lowering=False, debug=False)

    fp16 = mybir.dt.float16
    d_xt = nc.dram_tensor("xt", (128, RC * 8 * T), fp16, kind="ExternalInput")
    d_negxs = nc.dram_tensor("negxs", (G, T), fp16, kind="ExternalInput")
    d_qw16 = nc.dram_tensor(
        "qw16", (D, 128, RC * 1024), mybir.dt.int16, kind="ExternalInput"
    )
    d_s2 = nc.dram_tensor("s2", (D, 128, RC * 1024), fp16, kind="ExternalInput")
    d_wb = nc.dram_tensor("wb", (RC, 128, 8 * NC_OUT), fp16, kind="ExternalInput")
    d_sz = nc.dram_tensor("sz", (G, D * NC_OUT), fp16, kind="ExternalInput")
    d_out = nc.dram_tensor("out", (D, T, NC_OUT), mybir.dt.float32,
                           kind="ExternalOutput")

    AT = mybir.AluOpType
    FD = RC * 1024

    with tile.TileContext(nc) as tc:
        with (
            tc.tile_pool(name="const", bufs=1) as cpool,
            tc.tile_pool(name="qw", bufs=2) as qpool,
            tc.tile_pool(name="s2", bufs=2) as spool,
            tc.tile_pool(name="vr", bufs=1) as vrpool,
            tc.tile_pool(name="v", bufs=2) as vpool,
            tc.tile_pool(name="wb", bufs=2) as wpool,
            tc.tile_pool(name="outp", bufs=4) as opool,
            tc.tile_pool(name="ps", bufs=2, space="PSUM") as ppool,
            tc.tile_pool(name="psb", bufs=1, space="PSUM") as pbpool,
        ):
            xt_sb = cpool.tile([128, RC * 8 * T], fp16)
            negxs_sb = cpool.tile([G, T], fp16)
            sz_sb = cpool.tile([G, D * NC_OUT], fp16)
            base_sb = cpool.tile([128, 2 * NC_OUT], fp16)
            ident = cpool.tile([128, 128], fp16)

            # xt chunked per rc so the first matmuls start early
            for rc in range(RC):
                nc.sync.dma_start(xt_sb[:, rc * 8 * T:(rc + 1) * 8 * T],
                                  d_xt[:, rc * 8 * T:(rc + 1) * 8 * T])
            nc.sync.dma_start(negxs_sb[:], d_negxs[:])
            nc.sync.dma_start(sz_sb[:], d_sz[:])
            make_identity(nc, ident[:])

            def xt_tile(rc, j, th):
                off = (rc * 8 + j) * T + th * 128
                return xt_sb[:, off:off + 128]

            def adapter_main(d, ps):
                """Dequant (4 TS + 4 TT over [128, 4096]) + 64 matmuls."""
                qw_t = qpool.tile([128, FD], mybir.dt.int16, name="qw_t")
                nc.sync.dma_start(qw_t[:], d_qw16[d, :, :])
                s2_t = spool.tile([128, FD], fp16, name="s2_t")
                nc.sync.dma_start(s2_t[:], d_s2[d, :, :])
                for sh in range(4):
                    vr = vrpool.tile([128, FD], mybir.dt.int16,
                                     tag=f"vr{sh}", name=f"vr{sh}")
                    nc.vector.tensor_scalar(
                        out=vr[:], in0=qw_t[:],
                        scalar1=4 * sh, scalar2=0xF,
                        op0=AT.logical_shift_right, op1=AT.bitwise_and,
                    )
                    v = vpool.tile([128, FD], fp16, tag=f"v{sh}", name=f"v{sh}")
                    nc.vector.tensor_tensor(
                        out=v[:], in0=vr[:], in1=s2_t[:], op=AT.mult
                    )
                    for e in range(2):          # j = 4*e + sh
                        j = 4 * e + sh
                        for rc in range(RC):
                            base_off = rc * 1024 + e
                            rhs = v[:, base_off:(rc + 1) * 1024:2]
                            for th in range(2):
                                nc.tensor.matmul(
                                    ps[th][:],
                                    lhsT=xt_tile(rc, j, th),
                                    rhs=rhs,
                                    start=(sh == 0 and e == 0 and rc == 0),
                                    stop=False,
                                )

            def adapter_finish(d, ps):
                """Zeros correction + base add + PSUM drain + DMA out."""
                for th in range(2):
                    nc.tensor.matmul(
                        ps[th][:],
                        lhsT=negxs_sb[:, th * 128:(th + 1) * 128],
                        rhs=sz_sb[:, d * NC_OUT:(d + 1) * NC_OUT],
                        start=False, stop=False,
                    )
                    nc.tensor.matmul(
                        ps[th][:],
                        lhsT=ident[:],
                        rhs=base_sb[:, th * NC_OUT:(th + 1) * NC_OUT],
                        start=False, stop=True,
                    )
                for th in range(2):
                    o_t = opool.tile([128, NC_OUT], mybir.dt.float32, name="o_t")
                    nc.scalar.copy(o_t[:], ps[th][:])
                    nc.sync.dma_start(
                        d_out[d, th * 128:(th + 1) * 128, :], o_t[:]
                    )

            # ---- adapter 0 main first (PE starts as soon as V(0,0) ready),
            # then base (wb DMAs overlap adapter 0's dequant), then the rest.
            ps0 = [ppool.tile([128, NC_OUT], mybir.dt.float32, tag=f"ps{t}",
                              name=f"ps{t}") for t in range(2)]
            adapter_main(0, ps0)

            ps_b = [pbpool.tile([128, NC_OUT], mybir.dt.float32, tag=f"psb{t}",
                                name=f"psb{t}") for t in range(2)]
            for rc in range(RC):
                wb_t = wpool.tile([128, 8 * NC_OUT], fp16, name="wb_t")
                nc.sync.dma_start(wb_t[:], d_wb[rc, :, :])
                for j in range(8):
                    rhs = wb_t[:, j * NC_OUT:(j + 1) * NC_OUT]
                    for th in range(2):
                        nc.tensor.matmul(
                            ps_b[th][:],
                            lhsT=xt_tile(rc, j, th),
                            rhs=rhs,
                            start=(rc == 0 and j == 0),
                            stop=(rc == RC - 1 and j == 7),
                        )
            for th in range(2):
                nc.scalar.copy(base_sb[:, th * NC_OUT:(th + 1) * NC_OUT],
                               ps_b[th][:])

            adapter_finish(0, ps0)

            for d in range(1, D):
                ps = [ppool.tile([128, NC_OUT], mybir.dt.float32, tag=f"ps{t}",
                                 name=f"ps{t}") for t in range(2)]
                adapter_main(d, ps)
                adapter_finish(d, ps)

    nc.compile()
    return nc


def _prep_inputs(x, base_weight, qweight, qzeros, scales):
    """Host-side layout prep. Returns list of 8 per-core input maps."""
    x = np.asarray(x, dtype=np.float32)
    base_weight = np.asarray(base_weight, dtype=np.float32)
    qweight = np.asarray(qweight, dtype=np.int32)
    qzeros = np.asarray(qzeros, dtype=np.int32)
    scales = np.asarray(scales, dtype=np.float32)

    # stationary x tiles: xt[p, (rc*8+j)*T + t] = x[t, 8*(128*rc+p)+j]
    xr = np.ascontiguousarray(x.T).reshape(RC, 128, 8, T)        # [rc, p, j, t]
    xt = np.ascontiguousarray(xr.transpose(1, 0, 2, 3)).reshape(128, RC * 8 * T)
    xt = xt.astype(np.float16)

    # group sums of x (for the zeros-correction contraction), negated
    xs = x.reshape(T, G, GROUP).sum(axis=2)                       # [t, g]
    negxs = np.ascontiguousarray((-xs.T)).astype(np.float16)      # [g, t]

    # unpack qzeros (packed along out cols): z4[d, g, 8m+jj]
    jj = 4 * np.arange(8, dtype=np.int32)
    z4 = ((qzeros[:, :, :, None] >> jj[None, None, None, :]) & 0xF)
    z4 = z4.reshape(D, G, OUT)                                    # [d, g, n]
    sz_full = scales * (z4 + 1).astype(np.float32)                # [d, g, n]

    in_maps = []
    for c in range(NCORES):
        ns = slice(c * NC_OUT, (c + 1) * NC_OUT)

        # packed weights: [d, p, rc*1024 + h], h = 2n + e (int32 -> 2x int16)
        qw_c = np.ascontiguousarray(qweight[:, :, ns])            # [D, 512, 512]
        qw_c = qw_c.reshape(D, RC, 128, NC_OUT).transpose(0, 2, 1, 3)
        qw16 = np.ascontiguousarray(qw_c).view(np.int16)          # [D,128,RC,1024]
        qw16 = qw16.reshape(D, 128, RC * 1024)

        # scale tile: s2[d, p, rc*1024 + 2n+e] = s[d, 8rc + p//16, ns][n]
        s_c = scales[:, :, ns]                                    # [D, G, 512]
        s2 = s_c.reshape(D, RC, 8, NC_OUT)                        # [d, rc, gg, n]
        s2 = np.repeat(s2, 16, axis=2)                            # [d, rc, 128, n]
        s2 = np.repeat(s2, 2, axis=3).astype(np.float16)          # [d,rc,128,2n]
        s2 = np.ascontiguousarray(s2.transpose(0, 2, 1, 3)).reshape(
            D, 128, RC * 1024)

        bw_c = base_weight[ns, :]                                 # [512, 4096]
        wb = np.ascontiguousarray(bw_c.T).reshape(RC, 128, 8, NC_OUT)
        wb = wb.reshape(RC, 128, 8 * NC_OUT).astype(np.float16)

        sz_c = sz_full[:, :, ns]                                  # [D, G, 512]
        sz = np.ascontiguousarray(sz_c.transpose(1, 0, 2)).reshape(G, D * NC_OUT)
        sz = sz.astype(np.float16)

        in_maps.append({
            "xt": xt, "negxs": negxs,
            "qw16": np.ascontiguousarray(qw16),
            "s2": np.ascontiguousarray(s2),
            "wb": np.ascontiguousarray(wb),
            "sz": sz,
        })
    return in_maps


def _run(in_maps, trace=False):
    from concourse import bass_utils
    if "nc" not in _PROGRAM_CACHE:
        _PROGRAM_CACHE["nc"] = _build_program()
    nc = _PROGRAM_CACHE["nc"]
    res = bass_utils.run_bass_kernel_spmd(
        nc, in_maps, core_ids=list(range(NCORES)), trace=trace
    )
    return res


def kernel(x, base_weight, qweight, qzeros, scales, g_idx, _trace=False,
           _return_results=False):
    in_maps = _prep_inputs(x, base_weight, qweight, qzeros, scales)
    res = _run(in_maps, trace=_trace)
    out = np.concatenate([res.results[c]["out"] for c in range(NCORES)], axis=2)
    if _return_results:
        return out, res
    return out


# revision 12
# speedup vs baseline: 1.0736x; 1.0736x over previous
"""ColumnParallelLinearWithDelta: GPTQ-int4 LoRA-delta matmul on 8 trn2 cores.

out[d] = x @ dequant(qweight[d], qzeros[d], scales[d]) + x @ base_weight.T

Sharding: column-parallel — out_features (4096) split into 8 slices of 512,
one per NeuronCore; x replicated. Each core computes its [8, 256, 512] slice.

Math (per core, out-col slice ns):
  W[k, n]  = s[g(k), n] * (w4[k, n] - (z4[g(k), n] + 1)),  g(k) = k // 128
  delta    = x @ W = x @ (s .* w4)  -  xs @ (s .* (z4 + 1))
  with xs[t, g] = sum_{k in g} x[t, k]   (host-precomputed group sums)
  out[d]   = delta_d + base,  base = x @ base_weight[ns, :].T

Device pipeline per adapter d (all 4 row-chunks rc batched in one free dim):
  - DMA packed qweight as int16 [128, 4096] (free = rc*1024 + 2n + e)
  - 4x tensor_scalar (>> 4*sh) & 0xF -> int16 nibble planes (4x DVE mode;
    int16 halfword h=2n+e holds nibbles j = 4e + sh)
  - 4x tensor_tensor multiply by scale tile s2 (partition-replicated, x2
    free-interleaved; host-prepped) -> scaled fp16 weights (2x DVE mode)
  - 64 matmuls sh-major (pipelines with dequant): stationary xT tile for
    sub-chunk (rc, j=4e+sh) x moving plane slice (stride-2 free AP),
    accumulating into PSUM f32 per t-half
Then per adapter: K=32 correction matmul (-xs^T x sz) and an identity
matmul adding the shared base output; ScalarE copies PSUM->SBUF, DMA out.
"""

import numpy as np

# ---- problem constants (hardcoded; kernel.py must be self-contained) ----
T = 256          # tokens
IN = 4096        # in_features
OUT = 4096       # out_features
D = 8            # adapters
GROUP = 128      # quant group size
G = IN // GROUP  # 32 groups
NCORES = 8
NC_OUT = OUT // NCORES   # 512 out cols per core
RC = 4                   # row chunks of 128 packed int32 rows (512 rows total)

_PROGRAM_CACHE: dict = {}


def _build_program():
    import concourse.bacc as bacc
    import concourse.mybir as mybir
    import concourse.tile as tile

    nc = bacc.Bacc("TRN2", target_bir_lowering=False, debug=False)

    fp16 = mybir.dt.float16
    d_xt = nc.dram_tensor("xt", (128, RC * 8 * T), fp16, kind="ExternalInput")
    d_negxs = nc.dram_tensor("negxs", (G, T), fp16, kind="ExternalInput")
    d_qw16 = nc.dram_tensor(
        "qw16", (D, 128, RC * 1024), mybir.dt.int16, kind="ExternalInput"
    )
    d_s2 = nc.dram_tensor("s2", (D, 128, RC * 1024), fp16, kind="ExternalInput")
    d_wb = nc.dram_tensor("wb", (RC, 128, 8 * NC_OUT), fp16, kind="ExternalInput")
    d_sz = nc.dram_tensor("sz", (G, D * NC_OUT), fp16, kind="ExternalInput")
    d_out = nc.dram_tensor("out", (D, T, NC_OUT), mybir.dt.float32,
                           kind="ExternalOutput")

    AT = mybir.AluOpType
    FD = RC * 1024

    with tile.TileContext(nc) as tc:
        with (
            tc.tile_pool(name="const", bufs=1) as cpool,
            tc.tile_pool(name="qw", bufs=2) as qpool,
            tc.tile_pool(name="s2", bufs=2) as spool,
            tc.tile_pool(name="vr", bufs=1) as vrpool,
            tc.tile_pool(name="v", bufs=2) as vpool,
            tc.tile_pool(name="wb", bufs=2) as wpool,
            tc.tile_pool(name="outp", bufs=4) as opool,
            tc.tile_pool(name="ps", bufs=2, space="PSUM") as ppool,
            tc.tile_pool(name="psb", bufs=1, space="PSUM") as pbpool,
        ):
            xt_sb = cpool.tile([128, RC * 8 * T], fp16)
            negxs_sb = cpool.tile([G, T], fp16)
            sz_sb = cpool.tile([G, D * NC_OUT], fp16)
            base_sb = cpool.tile([128, 2 * NC_OUT], fp16)

            def xt_tile(rc, j, th):
                off = (rc * 8 + j) * T + th * 128
                return xt_sb[:, off:off + 128]

            def adapter_main(d, ps, prologue=None):
                """Dequant (4 TS + 4 TT over [128, 4096]) + 64 matmuls."""
                qw_t = qpool.tile([128, FD], mybir.dt.int16, name="qw_t")
                nc.sync.dma_start(qw_t[:], d_qw16[d, :, :])
                s2_t = spool.tile([128, FD], fp16, name="s2_t")
                nc.sync.dma_start(s2_t[:], d_s2[d, :, :])
                if prologue is not None:
                    prologue()
                for sh in range(4):
                    vr = vrpool.tile([128, FD], mybir.dt.int16,
                                     tag=f"vr{sh}", name=f"vr{sh}")
                    nc.vector.tensor_scalar(
                        out=vr[:], in0=qw_t[:],
                        scalar1=4 * sh, scalar2=0xF,
                        op0=AT.logical_shift_right, op1=AT.bitwise_and,
                    )
                    v = vpool.tile([128, FD], fp16, tag=f"v{sh}", name=f"v{sh}")
                    nc.vector.tensor_tensor(
                        out=v[:], in0=vr[:], in1=s2_t[:], op=AT.mult
                    )
                    for e in range(2):          # j = 4*e + sh
                        j = 4 * e + sh
                        for rc in range(RC):
                            base_off = rc * 1024 + e
                            rhs = v[:, base_off:(rc + 1) * 1024:2]
                            for th in range(2):
                                nc.tensor.matmul(
                                    ps[th][:],
                                    lhsT=xt_tile(rc, j, th),
                                    rhs=rhs,
                                    start=(sh == 0 and e == 0 and rc == 0),
                                    stop=False,
                                )

            def adapter_finish(d, ps):
                """Zeros correction; DVE drains PSUM while adding base."""
                for th in range(2):
                    nc.tensor.matmul(
                        ps[th][:],
                        lhsT=negxs_sb[:, th * 128:(th + 1) * 128],
                        rhs=sz_sb[:, d * NC_OUT:(d + 1) * NC_OUT],
                        start=False, stop=True,
                    )
                for th in range(2):
                    o_t = opool.tile([128, NC_OUT], mybir.dt.float32, name="o_t")
                    nc.vector.tensor_tensor(
                        out=o_t[:], in0=ps[th][:],
                        in1=base_sb[:, th * NC_OUT:(th + 1) * NC_OUT],
                        op=AT.add,
                    )
                    nc.sync.dma_start(
                        d_out[d, th * 128:(th + 1) * 128, :], o_t[:]
                    )

            # ---- adapter 0 main first (qw/s2 DMAs lead so DVE starts ~3us
            # in; xt/const DMAs queue right behind), then base (wb DMAs
            # overlap adapter 0's dequant), then the rest.
            def startup_dmas():
                for rc in range(RC):
                    nc.sync.dma_start(xt_sb[:, rc * 8 * T:(rc + 1) * 8 * T],
                                      d_xt[:, rc * 8 * T:(rc + 1) * 8 * T])
                nc.sync.dma_start(negxs_sb[:], d_negxs[:])
                nc.sync.dma_start(sz_sb[:], d_sz[:])

            ps0 = [ppool.tile([128, NC_OUT], mybir.dt.float32, tag=f"ps{t}",
                              name=f"ps{t}") for t in range(2)]
            adapter_main(0, ps0, prologue=startup_dmas)

            ps_b = [pbpool.tile([128, NC_OUT], mybir.dt.float32, tag=f"psb{t}",
                                name=f"psb{t}") for t in range(2)]
            for rc in range(RC):
                wb_t = wpool.tile([128, 8 * NC_OUT], fp16, name="wb_t")
                nc.sync.dma_start(wb_t[:], d_wb[rc, :, :])
                for j in range(8):
                    rhs = wb_t[:, j * NC_OUT:(j + 1) * NC_OUT]
                    for th in range(2):
                        nc.tensor.matmul(
                            ps_b[th][:],
                            lhsT=xt_tile(rc, j, th),
                            rhs=rhs,
                            start=(rc == 0 and j == 0),
                            stop=(rc == RC - 1 and j == 7),
                        )
            for th in range(2):
                nc.scalar.copy(base_sb[:, th * NC_OUT:(th + 1) * NC_OUT],
                               ps_b[th][:])

            adapter_finish(0, ps0)

            for d in range(1, D):
                ps = [ppool.tile([128, NC_OUT], mybir.dt.float32, tag=f"ps{t}",
                                 name=f"ps{t}") for t in range(2)]
                adapter_main(d, ps)
                adapter_finish(d, ps)

    nc.compile()
    return nc


def _prep_inputs(x, base_weight, qweight, qzeros, scales):
    """Host-side layout prep. Returns list of 8 per-core input maps."""
    x = np.asarray(x, dtype=np.float32)
    base_weight = np.asarray(base_weight, dtype=np.float32)
    qweight = np.asarray(qweight, dtype=np.int32)
    qzeros = np.asarray(qzeros, dtype=np.int32)
    scales = np.asarray(scales, dtype=np.float32)

    # stationary x tiles: xt[p, (rc*8+j)*T + t] = x[t, 8*(128*rc+p)+j]
    xr = np.ascontiguousarray(x.T).reshape(RC, 128, 8, T)        # [rc, p, j, t]
    xt = np.ascontiguousarray(xr.transpose(1, 0, 2, 3)).reshape(128, RC * 8 * T)
    xt = xt.astype(np.float16)

    # group sums of x (for the zeros-correction contraction), negated
    xs = x.reshape(T, G, GROUP).sum(axis=2)                       # [t, g]
    negxs = np.ascontiguousarray((-xs.T)).astype(np.float16)      # [g, t]

    # unpack qzeros (packed along out cols): z4[d, g, 8m+jj]
    jj = 4 * np.arange(8, dtype=np.int32)
    z4 = ((qzeros[:, :, :, None] >> jj[None, None, None, :]) & 0xF)
    z4 = z4.reshape(D, G, OUT)                                    # [d, g, n]
    sz_full = scales * (z4 + 1).astype(np.float32)                # [d, g, n]

    in_maps = []
    for c in range(NCORES):
        ns = slice(c * NC_OUT, (c + 1) * NC_OUT)

        # packed weights: [d, p, rc*1024 + h], h = 2n + e (int32 -> 2x int16)
        qw_c = np.ascontiguousarray(qweight[:, :, ns])            # [D, 512, 512]
        qw_c = qw_c.reshape(D, RC, 128, NC_OUT).transpose(0, 2, 1, 3)
        qw16 = np.ascontiguousarray(qw_c).view(np.int16)          # [D,128,RC,1024]
        qw16 = qw16.reshape(D, 128, RC * 1024)

        # scale tile: s2[d, p, rc*1024 + 2n+e] = s[d, 8rc + p//16, ns][n]
        s_c = scales[:, :, ns]                                    # [D, G, 512]
        s2 = s_c.reshape(D, RC, 8, NC_OUT)                        # [d, rc, gg, n]
        s2 = np.repeat(s2, 16, axis=2)                            # [d, rc, 128, n]
        s2 = np.repeat(s2, 2, axis=3).astype(np.float16)          # [d,rc,128,2n]
        s2 = np.ascontiguousarray(s2.transpose(0, 2, 1, 3)).reshape(
            D, 128, RC * 1024)

        bw_c = base_weight[ns, :]                                 # [512, 4096]
        wb = np.ascontiguousarray(bw_c.T).reshape(RC, 128, 8, NC_OUT)
        wb = wb.reshape(RC, 128, 8 * NC_OUT).astype(np.float16)

        sz_c = sz_full[:, :, ns]                                  # [D, G, 512]
        sz = np.ascontiguousarray(sz_c.transpose(1, 0, 2)).reshape(G, D * NC_OUT)
        sz = sz.astype(np.float16)

        in_maps.append({
            "xt": xt, "negxs": negxs,
            "qw16": np.ascontiguousarray(qw16),
            "s2": np.ascontiguousarray(s2),
            "wb": np.ascontiguousarray(wb),
            "sz": sz,
        })
    return in_maps


def _run(in_maps, trace=False):
    from concourse import bass_utils
    if "nc" not in _PROGRAM_CACHE:
        _PROGRAM_CACHE["nc"] = _build_program()
    nc = _PROGRAM_CACHE["nc"]
    res = bass_utils.run_bass_kernel_spmd(
        nc, in_maps, core_ids=list(range(NCORES)), trace=trace
    )
    return res


def kernel(x, base_weight, qweight, qzeros, scales, g_idx, _trace=False,
           _return_results=False):
    in_maps = _prep_inputs(x, base_weight, qweight, qzeros, scales)
    res = _run(in_maps, trace=_trace)
    out = np.concatenate([res.results[c]["out"] for c in range(NCORES)], axis=2)
    if _return_results:
        return out, res
    return out
